# revision 1
# baseline (speedup 1.0000x reference)
"""CentroidInstanceLoss on 8 Trainium2 NeuronCores (Bass/Tile), v4.

Sharding: BY SUBBATCH — core c owns all points of subbatch c (padded with
inert dummy points to a fixed tile count), so centroid tables are fully
local and no collectives are needed. The host pre-normalizes x and ships it
in fp8 along with both one-hot layouts (fp8 point-major for the segment-sum
pass, f16 segment-major for the gather pass) and per-point pull weights.
The device is a pure matmul/reduce pipeline:

  pass1:  sums = sum_t oh_t^T @ xn_t          (fp8 matmul, PSUM-accumulated)
  mid:    mu = sums * (1/counts)              (one DVE op)
  push:   q[:,k] = ||mu - rot_k(mu)||_1 via 32 rotation matmuls, reduced
          on-device against host-computed pair weights
  pass2:  diff_t = ohT_t^T @ mu - xn_t        (two matmuls into PSUM)
          d1 = abs-reduce(diff)               (DVE/Act split)
  end:    pull = ones^T @ (relu(d1-dv)^2 * w) reduced to one scalar

Output per core: [1, 2] f32 = (pull, push); host sums and divides by N.
"""

import numpy as np

import concourse.bass as bass
import concourse.bacc as bacc
import concourse.mybir as mybir
import concourse.tile as tile

f32 = mybir.dt.float32
f16 = mybir.dt.float16
f8 = mybir.dt.float8e4

# Problem shape (hardcoded per contract).
N_TOTAL = 262144
D = 256
S = 8
L = 64
NCORES = 8
DELTA_V = 0.5
DELTA_D = 1.5
T_PAD = 264           # tiles of 128 points per core (33792 >= max subbatch)
NPC = T_PAD * 128     # padded points per core
KROT = 32             # push rotations (k and 64-k mirror; host doubles k<32)
ACT_REDUCE_MOD = 3    # batches with a % MOD == MOD-1 reduce on Act instead of DVE

AluOp = mybir.AluOpType
ActFn = mybir.ActivationFunctionType
Axis = mybir.AxisListType

f8_np = mybir.dt.np(f8)


def build_nc(n_core: int = NPC, use_collectives: bool = True, reps: int = 1,
             phases: tuple = ("p1", "push", "p2")):
    """SPMD program for one core (n_core/use_collectives/reps kept for API
    compat with tooling; ignored)."""
    T = T_PAD
    nc = bacc.Bacc("TRN2", target_bir_lowering=False, debug=False,
                   num_devices=NCORES)

    xn_in = nc.dram_tensor("xn", [128, T, D], f8, kind="ExternalInput")
    oh_in = nc.dram_tensor("oh", [128, T, L], f8, kind="ExternalInput")
    ohT_in = nc.dram_tensor("ohT", [L, T, 128], f16, kind="ExternalInput")
    wpt_in = nc.dram_tensor("wpt", [128, T], f32, kind="ExternalInput")
    negident_in = nc.dram_tensor("negident", [128, 128], f8, kind="ExternalInput")
    perms_in = nc.dram_tensor("perms", [L, KROT, L], f16, kind="ExternalInput")
    pushw_in = nc.dram_tensor("pushw", [L, KROT + 1], f32, kind="ExternalInput")
    crecip_in = nc.dram_tensor("crecip", [L, 1], f32, kind="ExternalInput")
    ones128_in = nc.dram_tensor("ones128", [128, 1], f16, kind="ExternalInput")
    ones64_in = nc.dram_tensor("ones64", [L, 1], f16, kind="ExternalInput")

    res_out = nc.dram_tensor("res", [1, 2], f32, kind="ExternalOutput")

    NCH = 8            # DMA chunks
    CH = T // NCH

    with tile.TileContext(nc) as tc:
        with (
            tc.tile_pool(name="const", bufs=1) as constp,
            tc.tile_pool(name="big", bufs=1) as bigp,
            tc.tile_pool(name="small", bufs=1) as smallp,
        ):
            # ---- constants ----
            negident_sb = constp.tile([128, 128], f8)
            nc.sync.dma_start(negident_sb[:], negident_in[:])
            perms_sb = constp.tile([L, KROT, L], f16)
            nc.sync.dma_start(perms_sb[:], perms_in[:])
            pushw_sb = constp.tile([L, KROT + 1], f32)
            nc.sync.dma_start(pushw_sb[:], pushw_in[:])
            crecip_sb = constp.tile([L, 1], f32)
            nc.sync.dma_start(crecip_sb[:], crecip_in[:])
            ones128_sb = constp.tile([128, 1], f16)
            nc.sync.dma_start(ones128_sb[:], ones128_in[:])
            ones64_sb = constp.tile([L, 1], f16)
            nc.sync.dma_start(ones64_sb[:], ones64_in[:])
            wpt_sb = constp.tile([128, T], f32)
            nc.sync.dma_start(wpt_sb[:], wpt_in[:])
            negdv_sb = constp.tile([128, 1], f32)
            nc.vector.memset(negdv_sb[:], -DELTA_V)
            twodd_sb = constp.tile([L, 1], f32)
            nc.vector.memset(twodd_sb[:], 2.0 * DELTA_D)

            # ---- resident state ----
            xn_all = bigp.tile([128, T, D], f8, name="xn_all")
            oh_all = bigp.tile([128, T, L], f8, name="oh_all")
            d1_all = bigp.tile([128, T], f32, name="d1_all")
            q_sb = bigp.tile([L, KROT + 1], f32, name="q_sb")
            mu16 = bigp.tile([L, D], f16, name="mu16")
            res_sb = smallp.tile([1, 2], f32, name="res_sb")
            nc.vector.memset(d1_all[:], 0.0)
            nc.vector.memset(q_sb[:], 0.0)

            # ---- pass 1: local segment sums ----
            with tc.tile_pool(name="ps1", bufs=1, space="PSUM") as ps1:
                ps_sums = ps1.tile([L, D], f32, name="ps_sums")
                if "p1" in phases:
                    for c in range(NCH):
                        sl = slice(c * CH, (c + 1) * CH)
                        nc.sync.dma_start(xn_all[:, sl, :], xn_in[:, sl, :])
                        nc.sync.dma_start(oh_all[:, sl, :], oh_in[:, sl, :])
                    for t in range(T):
                        nc.tensor.matmul(
                            ps_sums[:],
                            oh_all[:, t, :],
                            xn_all[:, t, :],
                            start=(t == 0), stop=(t == T - 1),
                        )
                else:
                    nc.vector.memset(ps_sums[:], 0.0)

                # ---- mu = sums / counts ----
                nc.vector.tensor_scalar(
                    mu16[:], ps_sums[:], crecip_sb[:, 0:1], None, op0=AluOp.mult,
                )

            # ---- push ----
            with (
                tc.tile_pool(name="rotps", bufs=2, space="PSUM") as rotpsp,
                tc.tile_pool(name="pdp", bufs=3) as pdp,
            ):
                if "push" in phases:
                    for k in range(1, KROT + 1):
                        ps_rot = rotpsp.tile([L, D], f32, tag="rotps")
                        nc.tensor.matmul(
                            ps_rot[:], perms_sb[:, k - 1, :], mu16[:],
                            start=True, stop=True,
                        )
                        pd = pdp.tile([L, D], f16, tag="pd")
                        nc.vector.tensor_sub(pd[:], mu16[:], ps_rot[:])
                        psink = pdp.tile([L, D], f16, tag="psink")
                        nc.scalar.activation(
                            psink[:], pd[:], ActFn.Abs,
                            accum_out=q_sb[:, k:k + 1],
                        )
                rp = pdp.tile([L, KROT], f16, tag="rp")
                nc.scalar.activation(
                    rp[:], q_sb[:, 1:KROT + 1], ActFn.Relu,
                    bias=twodd_sb[:], scale=-1.0,
                )
                r2p = pdp.tile([L, KROT], f16, tag="r2p")
                nc.vector.tensor_mul(r2p[:], rp[:], rp[:])
                vp = pdp.tile([L, KROT], f16, tag="vp")
                nc.vector.tensor_mul(vp[:], r2p[:], pushw_sb[:, 1:KROT + 1])
                ps_push = rotpsp.tile([1, KROT], f32, tag="pushacc")
                nc.tensor.matmul(
                    ps_push[:], ones64_sb[:], vp[:], start=True, stop=True,
                )
                nc.vector.tensor_reduce(
                    res_sb[0:1, 1:2], ps_push[:], axis=Axis.X, op=AluOp.add,
                )

            # ---- pass 2: diff in PSUM (gather - xn), then abs-reduce ----
            with (
                tc.tile_pool(name="ohtp", bufs=2) as ohtp,
                tc.tile_pool(name="mups", bufs=2, space="PSUM") as mupsp,
                tc.tile_pool(name="sinkp", bufs=2) as sinkp,
            ):
                if "p2" in phases:
                    oht_bufs = []
                    for c in range(NCH):
                        ohtb = ohtp.tile([L, CH, 128], f16, tag="ohtb")
                        nc.sync.dma_start(
                            ohtb[:], ohT_in[:, c * CH:(c + 1) * CH, :]
                        )
                        oht_bufs.append(ohtb)
                    for a in range(T // 4):
                        ps_mu4 = mupsp.tile([128, 4, D], f32, tag="mu4")
                        for j in range(4):
                            t = 4 * a + j
                            ohtb = oht_bufs[t // CH]
                            nc.tensor.matmul(
                                ps_mu4[:, j, :],
                                ohtb[:, t % CH, :],
                                mu16[:],
                                start=True, stop=False,
                            )
                            nc.tensor.matmul(
                                ps_mu4[:, j, :],
                                negident_sb[:],
                                xn_all[:, t, :],
                                start=False, stop=True,
                            )
                        if a % ACT_REDUCE_MOD == ACT_REDUCE_MOD - 1:
                            for j in range(4):
                                t = 4 * a + j
                                sink = sinkp.tile([128, D], f16, tag="sink")
                                nc.scalar.activation(
                                    sink[:], ps_mu4[:, j, :], ActFn.Abs,
                                    accum_out=d1_all[:, t:t + 1],
                                )
                        else:
                            nc.vector.tensor_reduce(
                                d1_all[:, 4 * a:4 * a + 4], ps_mu4[:],
                                axis=Axis.X, op=AluOp.add,
                                apply_absolute_value=True,
                            )

                # ---- pull reduction ----
                r_all = sinkp.tile([128, T], f16, tag="r_all")
                nc.scalar.activation(
                    r_all[:], d1_all[:], ActFn.Relu, bias=negdv_sb[:],
                )
                r2_all = sinkp.tile([128, T], f16, tag="r2_all")
                nc.vector.tensor_mul(r2_all[:], r_all[:], r_all[:])
                v_all = sinkp.tile([128, T], f16, tag="v_all")
                nc.vector.tensor_mul(v_all[:], r2_all[:], wpt_sb[:])
                ps_pull = mupsp.tile([1, T], f32, tag="pullacc")
                nc.tensor.matmul(
                    ps_pull[:], ones128_sb[:], v_all[:], start=True, stop=True,
                )
                nc.vector.tensor_reduce(
                    res_sb[0:1, 0:1], ps_pull[:], axis=Axis.X, op=AluOp.add,
                )

            nc.sync.dma_start(res_out[:], res_sb[:])

    nc.compile()
    return nc


def host_tables(labels: np.ndarray, subbatch: np.ndarray):
    seg = (subbatch.astype(np.int64) * L + labels.astype(np.int64)).astype(np.int32)
    counts = np.bincount(seg, minlength=S * L).astype(np.float64)
    present = counts > 0
    M = present.reshape(S, L).sum(axis=1).astype(np.float64)
    valid = M > 1.0
    M_per_seg = np.repeat(M, L)
    valid_per_seg = np.repeat(valid, L)
    w = np.where(
        valid_per_seg, 1.0 / (M_per_seg * np.maximum(counts, 1.0)), 0.0
    ).astype(np.float32)
    crecip = (1.0 / np.maximum(counts, 1.0)).astype(np.float32)
    return seg, counts, present, M, valid, w, crecip


def make_in_maps(outputs: np.ndarray, labels: np.ndarray, subbatch: np.ndarray):
    n = outputs.shape[0]
    seg, counts, present, M, valid, w, crecip = host_tables(labels, subbatch)

    x = outputs.astype(np.float32)
    nrm = np.sqrt((x * x).sum(axis=1)) + 1e-8
    xn = (x / nrm[:, None]).astype(f8_np)

    order = np.argsort(subbatch, kind="stable")
    sb_sizes = np.bincount(subbatch, minlength=S)
    assert sb_sizes.max() <= NPC, f"subbatch overflow: {sb_sizes.max()} > {NPC}"

    negident = (-np.eye(128)).astype(f8_np)
    pp, kk, mm = np.meshgrid(
        np.arange(L), np.arange(1, KROT + 1), np.arange(L), indexing="ij")
    perms = (pp == (mm + kk) % L).astype(np.float16)
    ones128 = np.ones((128, 1), np.float16)
    ones64 = np.ones((L, 1), np.float16)

    pres_sl = present.reshape(S, L)
    in_maps = []
    starts = np.concatenate([[0], np.cumsum(sb_sizes)])
    for c in range(NCORES):
        idx = order[starts[c]:starts[c + 1]]
        m = idx.shape[0]
        xn_c = np.zeros((NPC, D), dtype=f8_np)
        xn_c[:m] = xn[idx]
        oh_c = np.zeros((NPC, L), dtype=f8_np)
        oh_c[np.arange(m), labels[idx]] = 1.0
        w_c = np.zeros((NPC,), dtype=np.float32)
        w_c[:m] = w[seg[idx]]

        blk = slice(c * L, (c + 1) * L)
        crec_c = crecip[blk].reshape(L, 1)

        p = pres_sl[c]
        pw = np.zeros((L, KROT + 1), dtype=np.float32)
        if valid[c]:
            denom = max(M[c] * (M[c] - 1.0), 1.0)
            for k in range(1, KROT + 1):
                mask = p & np.roll(p, -k)  # p[a] & p[(a+k)%L]
                wk = 2.0 if k < KROT else 1.0
                pw[:, k] = mask.astype(np.float32) * wk / denom

        oh_t = oh_c.reshape(T_PAD, 128, L)
        in_maps.append({
            "xn": np.ascontiguousarray(
                xn_c.reshape(T_PAD, 128, D).transpose(1, 0, 2)),
            "oh": np.ascontiguousarray(oh_t.transpose(1, 0, 2)),
            "ohT": np.ascontiguousarray(
                oh_t.transpose(2, 0, 1).astype(np.float16)),
            "wpt": np.ascontiguousarray(w_c.reshape(T_PAD, 128).T),
            "negident": negident,
            "perms": perms,
            "pushw": pw,
            "crecip": crec_c,
            "ones128": ones128,
            "ones64": ones64,
        })
    tables = (seg, counts, present, M, valid, w, crecip)
    return in_maps, tables


def combine(results, tables, n: int):
    total = np.float64(0.0)
    for r in results:
        total += np.asarray(r["res"], dtype=np.float64).sum()
    return np.float32(total / n)


_NC_CACHE: dict = {}


def _get_nc(n_core: int = NPC):
    key = "v4"
    if key not in _NC_CACHE:
        _NC_CACHE[key] = build_nc(n_core)
    return _NC_CACHE[key]


def kernel(outputs, labels, subbatch_indices):
    from concourse.bass_utils import run_bass_kernel_spmd

    outputs = np.asarray(outputs, dtype=np.float32)
    labels = np.asarray(labels, dtype=np.int32)
    subbatch_indices = np.asarray(subbatch_indices, dtype=np.int32)
    n = outputs.shape[0]

    nc = _get_nc()
    in_maps, tables = make_in_maps(outputs, labels, subbatch_indices)
    res = run_bass_kernel_spmd(nc, in_maps, list(range(NCORES)))
    return combine(res.results, tables, n)



# revision 49
# speedup vs baseline: 14.1486x; 14.1486x over previous
"""CentroidInstanceLoss on 8 Trainium2 NeuronCores (Bass/Tile), v6.

Sharding: BY SUBBATCH — core c owns all points of subbatch c. Host sorts each
core's points by label and packs them into 127-point tiles (partitions 0..126),
padding each label to exactly G=5 tiles, so tile t belongs to label t//G.
Labels are independent, so the kernel pipelines per 8-label chunk:

  per chunk c (40 tiles, 8 labels):
    DMA chunk -> p1 segment sums (fp8 DoubleRow matmuls, one-hot-col weights)
    -> mu16 (DVE) + 16*mu fp8 (Act) -> broadcast-DMA 16*mu into partition 127
    -> p2: per tile ONE matmul diff = W^T @ xn_tile (W = e127/16 - I, fp8)
       abs-reduce split across DVE / Act / Pool into per-engine d1 tensors
  push: 16 stacked (I-P) matmuls into spare PSUM groups, reduced like p2
  finale: pull = ones^T @ (relu(d1-dv)^2 * w); push term likewise.

reps>1 repeats the whole body (incl. DMA) for clean on-device timing.
"""

import numpy as np

import concourse.bass as bass
import concourse.bacc as bacc
import concourse.mybir as mybir
import concourse.tile as tile

f32 = mybir.dt.float32
f16 = mybir.dt.float16
f8 = mybir.dt.float8e4

# Problem shape (hardcoded per contract).
N_TOTAL = 262144
D = 256
S = 8
L = 64
NCORES = 8
DELTA_V = 0.5
DELTA_D = 1.5

G = 5                 # tiles per label (5*127 = 635 >= max label count)
PPT = 127             # points per tile (partition 127 holds 16*mu)
T = L * G             # 320 tiles per core
NPC = T * PPT         # padded points per core
KROT = 32             # push rotations (k and 64-k mirror; host doubles k<32)
NCH = 8               # chunks (8 labels / 40 tiles each)
LCH = L // NCH        # labels per chunk
CHT = T // NCH        # tiles per chunk (40)

# per-chunk reduce schedules: (engine, ntiles) — v=DVE grouped tensor_reduce
# (4-tile groups), a=Act abs+accum per tile. GPSIMD can run neither
# TensorScalarPtr nor free-axis reduces, so Pool sits this one out.
# Chunks alternate v28/a12 and v24/a16 to balance engine rates (~.32 vs .57
# us/tile), averaging v26/a14.
CHUNK_SCHED_EVEN = [
    ("a", 2), ("v", 4), ("a", 2), ("v", 4), ("a", 2), ("v", 4),
    ("a", 2), ("v", 4), ("a", 2), ("v", 4), ("a", 2), ("v", 4), ("v", 4),
]
CHUNK_SCHED_ODD = [
    ("a", 2), ("v", 4), ("a", 2), ("v", 4), ("a", 2), ("v", 4),
    ("a", 2), ("v", 4), ("a", 2), ("v", 4), ("a", 2), ("v", 4),
    ("a", 2), ("a", 2),
]

AluOp = mybir.AluOpType
ActFn = mybir.ActivationFunctionType
Axis = mybir.AxisListType
PerfMode = mybir.MatmulPerfMode

f8_np = mybir.dt.np(f8)


def build_nc(n_core: int = NPC, use_collectives: bool = True, reps: int = 1):
    assert sum(n for _, n in CHUNK_SCHED_EVEN) == CHT
    assert sum(n for _, n in CHUNK_SCHED_ODD) == CHT
    nc = bacc.Bacc("TRN2", target_bir_lowering=False, debug=False,
                   num_devices=NCORES)

    xn_in = nc.dram_tensor("xn", [PPT, T, D], f8, kind="ExternalInput")
    wpt_in = nc.dram_tensor("wpt", [128, T], f32, kind="ExternalInput")
    w2_in = nc.dram_tensor("w2", [128, 128], f8, kind="ExternalInput")
    # [k, label, 2, m]: global one-hot column duplicated for DoubleRow
    idrep_in = nc.dram_tensor("idrep", [PPT, L, 2, L], f8,
                              kind="ExternalInput")
    perms_in = nc.dram_tensor("perms", [L, KROT // 2, 128], f16,
                              kind="ExternalInput")
    pushw_in = nc.dram_tensor("pushw", [128, KROT // 2], f32,
                              kind="ExternalInput")
    crecip_in = nc.dram_tensor("crecip", [L, 1], f32, kind="ExternalInput")
    crecip16_in = nc.dram_tensor("crecip16", [L, 1], f32,
                                 kind="ExternalInput")

    res_out = nc.dram_tensor("res", [1, 2], f32, kind="ExternalOutput")

    with tile.TileContext(nc) as tc:
        with (
            tc.tile_pool(name="const", bufs=1) as constp,
            tc.tile_pool(name="big", bufs=1) as bigp,
        ):
            # ---- constants (loaded once) ----
            w2_sb = constp.tile([128, 128], f8)
            nc.sync.dma_start(w2_sb[:], w2_in[:])
            idrep_sb = constp.tile([PPT, L, 2, L], f8)
            nc.sync.dma_start(idrep_sb[:], idrep_in[:])
            perms_sb = constp.tile([L, KROT // 2, 128], f16)
            nc.sync.dma_start(perms_sb[:], perms_in[:])
            pushw_sb = constp.tile([128, KROT // 2], f32)
            nc.sync.dma_start(pushw_sb[:], pushw_in[:])
            crecip_sb = constp.tile([L, 1], f32)
            nc.sync.dma_start(crecip_sb[:], crecip_in[:])
            crecip16_sb = constp.tile([L, 1], f32)
            nc.sync.dma_start(crecip16_sb[:], crecip16_in[:])
            wpt_sb = constp.tile([128, T], f32)
            nc.sync.dma_start(wpt_sb[:], wpt_in[:])
            negdv_sb = constp.tile([128, 1], f32)
            nc.vector.memset(negdv_sb[:], -DELTA_V)
            twodd_sb = constp.tile([128, 1], f32)
            nc.vector.memset(twodd_sb[:], 2.0 * DELTA_D)
            ones128_sb = constp.tile([128, 1], f16)
            nc.vector.memset(ones128_sb[:], 1.0)

            # ---- resident state ----
            xn_all = bigp.tile([128, T, D], f8, name="xn_all")
            # one d1 tensor per reduce engine so writers never share a tile
            d1_eng = {
                "v": bigp.tile([128, T], f32, name="d1_v"),
                "a": bigp.tile([128, T], f32, name="d1_a"),
            }
            q2_sb = bigp.tile([128, KROT // 2], f32, name="q2_sb")
            mu64 = bigp.tile([L, D], f16, name="mu64")
            muf8 = bigp.tile([L, D], f8, name="muf8")
            res_sb = bigp.tile([1, 2], f32, name="res_sb")
            v_eng = {
                k: bigp.tile([128, T], f16, name=f"v_{k}") for k in "va"
            }
            vp_sb = bigp.tile([128, KROT // 2], f16, name="vp_sb")
            # zero once: each engine rewrites only its own columns every rep,
            # and untouched columns must stay 0 (relu(0-dv)=0 kills them)
            for dt in d1_eng.values():
                nc.vector.memset(dt[:], 0.0)

            for rep in range(reps):
                with (
                    tc.tile_pool(name="ps1", bufs=1, space="PSUM") as ps1p,
                    tc.tile_pool(name="psv", bufs=2, space="PSUM") as psv,
                    tc.tile_pool(name="psa", bufs=3, space="PSUM") as psa,
                    tc.tile_pool(name="sinkp", bufs=3) as sinkp,
                ):
                    pools = {"v": psv, "a": psa}

                    def consume(kind, psg, n, out_t, out_col):
                        """Reduce |psg[:, :n, :]| along D into
                        out_t[:, out_col:out_col+n] on engine `kind`."""
                        if kind == "v":
                            nc.vector.tensor_reduce(
                                out_t[:, out_col:out_col + n], psg[:],
                                axis=Axis.X, op=AluOp.add,
                                apply_absolute_value=True,
                            )
                        else:
                            for j in range(n):
                                sink = sinkp.tile([128, D], f16, tag="asink")
                                nc.scalar.activation(
                                    sink[:], psg[:, j, :], ActFn.Abs,
                                    accum_out=out_t[:, out_col + j:out_col + j + 1],
                                )

                    # bounded prefetch: keep ~3 chunk DMAs in flight so the
                    # per-chunk mu scatter is never stuck behind bulk input
                    PREFETCH = 3

                    def issue_chunk_dma(c):
                        if c < NCH:
                            s = slice(c * CHT, (c + 1) * CHT)
                            nc.sync.dma_start(xn_all[0:PPT, s, :],
                                              xn_in[:, s, :])

                    def p1_mu(c):
                        """Segment sums + mu + scatter for chunk c."""
                        tc0 = c * CHT
                        sl = slice(tc0, tc0 + CHT)
                        ps_c = ps1p.tile([L, D], f32, tag="ps_c")
                        lsl = slice(c * LCH, (c + 1) * LCH)
                        first = True
                        for li in range(LCH):
                            l = c * LCH + li
                            t0 = tc0 + li * G
                            lhs2 = idrep_sb[:, l, :, :]
                            nc.tensor.matmul(
                                ps_c[:], lhs2, xn_all[0:PPT, t0:t0 + 2, :],
                                start=first, stop=False,
                                perf_mode=PerfMode.DoubleRow,
                            )
                            first = False
                            nc.tensor.matmul(
                                ps_c[:], lhs2, xn_all[0:PPT, t0 + 2:t0 + 4, :],
                                start=False, stop=False,
                                perf_mode=PerfMode.DoubleRow,
                            )
                        # leftover 5th tiles: cross-label DoubleRow pairs
                        # idrep[:, l:l+2, 0, :] puts col l in slot 0 and
                        # col l+1 in slot 1
                        for i in range(LCH // 2):
                            l = c * LCH + 2 * i
                            base = tc0 + (2 * i) * G + (G - 1)
                            lhsx = idrep_sb[:, l:l + 2, 0, :]
                            nc.tensor.matmul(
                                ps_c[:], lhsx,
                                xn_all[0:PPT, base:base + G + 1:G, :],
                                start=False, stop=(i == LCH // 2 - 1),
                                perf_mode=PerfMode.DoubleRow,
                            )

                        # full base-0 [64, D] ops (engine ops must start at
                        # partition 0); rows outside this chunk are 0
                        mu_tmp = sinkp.tile([L, D], f16, tag="mutmp")
                        nc.vector.tensor_scalar(
                            mu_tmp[:], ps_c[:], crecip_sb[:, 0:1],
                            None, op0=AluOp.mult,
                        )
                        mu8_tmp = sinkp.tile([L, D], f8, tag="mu8tmp")
                        nc.scalar.activation(
                            mu8_tmp[:], ps_c[:], ActFn.Copy,
                            scale=crecip16_sb[:, 0:1],
                        )
                        # scatter 16*mu into partition 127 of chunk tiles
                        nc.sync.dma_start(
                            xn_all[127:128, sl, :],
                            mu8_tmp[lsl, :].unsqueeze(1).broadcast_to(
                                (LCH, G, D)),
                        )
                        # collect this chunk's mu rows for the push term
                        nc.sync.dma_start(mu64[lsl, :], mu_tmp[lsl, :])
                        issue_chunk_dma(c + PREFETCH)

                    def p2_chunk(c):
                        t = c * CHT
                        sched = CHUNK_SCHED_EVEN if c % 2 == 0 else CHUNK_SCHED_ODD
                        for kind, n in sched:
                            psg = pools[kind].tile([128, n, D], f32,
                                                   tag=f"mu_{kind}")
                            for j in range(n):
                                nc.tensor.matmul(
                                    psg[:, j, :], w2_sb[:], xn_all[:, t + j, :],
                                    start=True, stop=True,
                                )
                            consume(kind, psg, n, d1_eng[kind], t)
                            t += n

                    for c in range(PREFETCH):
                        issue_chunk_dma(c)
                    # software pipeline: p1/mu/scatter run one chunk ahead of
                    # p2, so the scatter DMA never stalls the PE between chunks
                    p1_mu(0)
                    for c in range(NCH):
                        if c + 1 < NCH:
                            p1_mu(c + 1)
                        p2_chunk(c)

                    # ---- push: 16 stacked (I-P) rotations through the same
                    # machinery, accumulated into q2 columns ----
                    jj = 0
                    for kind, n in (("v", 4), ("a", 2), ("a", 2),
                                    ("v", 4), ("a", 2), ("a", 2)):
                        psg = pools[kind].tile([128, n, D], f32,
                                               tag=f"mu_{kind}")
                        for j in range(n):
                            nc.tensor.matmul(
                                psg[:, j, :], perms_sb[:, jj + j, :], mu64[:],
                                start=True, stop=True,
                            )
                        consume(kind, psg, n, q2_sb, jj)
                        jj += n

                    # ---- pull + push elementwise ----
                    v_alls = []
                    for kind, d1t in d1_eng.items():
                        r_all = sinkp.tile([128, T], f16, tag=f"r_{kind}")
                        nc.scalar.activation(
                            r_all[:], d1t[:], ActFn.Relu, bias=negdv_sb[:],
                        )
                        r2_all = sinkp.tile([128, T], f16, tag=f"r2_{kind}")
                        nc.vector.tensor_mul(r2_all[:], r_all[:], r_all[:])
                        v_all = v_eng[kind]
                        nc.vector.tensor_mul(v_all[:], r2_all[:], wpt_sb[:])
                        v_alls.append(v_all)

                    rp = sinkp.tile([128, KROT // 2], f16, tag="rp")
                    nc.scalar.activation(
                        rp[:], q2_sb[:], ActFn.Relu,
                        bias=twodd_sb[:], scale=-1.0,
                    )
                    r2p = sinkp.tile([128, KROT // 2], f16, tag="r2p")
                    nc.vector.tensor_mul(r2p[:], rp[:], rp[:])
                    vp = vp_sb
                    nc.vector.tensor_mul(vp[:], r2p[:], pushw_sb[:])

                # ---- final dot products (own small PSUM pool) ----
                with tc.tile_pool(name="finps", bufs=1, space="PSUM") as finp:
                    ps_pull = finp.tile([1, T], f32, tag="pullacc")
                    for i, v_all in enumerate(v_alls):
                        nc.tensor.matmul(
                            ps_pull[:], ones128_sb[:], v_all[:],
                            start=(i == 0), stop=(i == len(v_alls) - 1),
                        )
                    nc.vector.tensor_reduce(
                        res_sb[0:1, 0:1], ps_pull[:], axis=Axis.X,
                        op=AluOp.add,
                    )
                    ps_push = finp.tile([1, KROT // 2], f32, tag="pushacc")
                    nc.tensor.matmul(
                        ps_push[:], ones128_sb[:], vp[:],
                        start=True, stop=True,
                    )
                    nc.vector.tensor_reduce(
                        res_sb[0:1, 1:2], ps_push[:], axis=Axis.X,
                        op=AluOp.add,
                    )

                nc.sync.dma_start(res_out[:], res_sb[:])

    nc.compile()
    return nc


def host_tables(labels: np.ndarray, subbatch: np.ndarray):
    seg = (subbatch.astype(np.int64) * L + labels.astype(np.int64)).astype(
        np.int32)
    counts = np.bincount(seg, minlength=S * L).astype(np.float64)
    present = counts > 0
    M = present.reshape(S, L).sum(axis=1).astype(np.float64)
    valid = M > 1.0
    M_per_seg = np.repeat(M, L)
    valid_per_seg = np.repeat(valid, L)
    w = np.where(
        valid_per_seg, 1.0 / (M_per_seg * np.maximum(counts, 1.0)), 0.0
    ).astype(np.float32)
    crecip = (1.0 / np.maximum(counts, 1.0)).astype(np.float32)
    return seg, counts, present, M, valid, w, crecip


def make_in_maps(outputs: np.ndarray, labels: np.ndarray, subbatch: np.ndarray):
    n = outputs.shape[0]
    seg, counts, present, M, valid, w, crecip = host_tables(labels, subbatch)

    x = outputs.astype(np.float32)
    nrm = np.sqrt((x * x).sum(axis=1)) + 1e-8
    xn = (x / nrm[:, None]).astype(f8_np)

    assert counts.max() <= G * PPT, (
        f"label overflow: {counts.max()} > {G * PPT}")

    # rank of each point within its (subbatch, label) segment
    order = np.argsort(seg, kind="stable")
    seg_sorted = seg[order]
    starts_per_seg = np.concatenate(
        [[0], np.cumsum(np.bincount(seg, minlength=S * L))])
    rank_sorted = np.arange(n) - starts_per_seg[seg_sorted]
    rank = np.empty(n, dtype=np.int64)
    rank[order] = rank_sorted

    core = subbatch.astype(np.int64)
    lab = labels.astype(np.int64)
    p_idx = rank % PPT
    t_idx = lab * G + rank // PPT

    xn_arr = np.zeros((NCORES, PPT, T, D), dtype=f8_np)
    xn_arr[core, p_idx, t_idx] = xn
    wpt_arr = np.zeros((NCORES, 128, T), dtype=np.float32)
    wpt_arr[core, p_idx, t_idx] = w[seg]

    # constant weight matrix: out[m] = xn[127]/16 - xn[m]
    w2 = -np.eye(128, dtype=np.float32)
    w2[127, :] += 1.0 / 16.0
    w2 = w2.astype(f8_np)

    # one-hot-column p1 weights: idrep[k, l, i, m] = (m == l), DoubleRow-dup
    idrep = np.broadcast_to(
        np.eye(L, dtype=np.float32)[None, :, None, :], (PPT, L, 2, L)
    ).astype(f8_np)

    # push: stacked (I - P_k) pairs; rows 0..63 -> k=2j+1, 64..127 -> k=2j+2
    perms = np.zeros((L, KROT // 2, 128), dtype=np.float16)
    a = np.arange(L)
    for j in range(KROT // 2):
        for half, k in ((0, 2 * j + 1), (1, 2 * j + 2)):
            P = np.zeros((L, L), dtype=np.float32)
            P[(a + k) % L, a] = 1.0   # P[p, m]: rot[m] = mu[(m+k)%L]
            mat = np.eye(L, dtype=np.float32) - P
            perms[:, j, half * L:(half + 1) * L] = mat.astype(np.float16)

    pres_sl = present.reshape(S, L)
    in_maps = []
    for c in range(NCORES):
        blk = slice(c * L, (c + 1) * L)
        crec_c = crecip[blk].reshape(L, 1)

        p = pres_sl[c]
        pw = np.zeros((128, KROT // 2), dtype=np.float32)
        if valid[c]:
            denom = max(M[c] * (M[c] - 1.0), 1.0)
            for j in range(KROT // 2):
                for half, k in ((0, 2 * j + 1), (1, 2 * j + 2)):
                    mask = p & np.roll(p, -k)  # p[m] & p[(m+k)%L]
                    wk = 2.0 if k < KROT else 1.0
                    pw[half * L:(half + 1) * L, j] = (
                        mask.astype(np.float32) * wk / denom)

        in_maps.append({
            "xn": np.ascontiguousarray(xn_arr[c]),
            "wpt": np.ascontiguousarray(wpt_arr[c]),
            "w2": w2,
            "idrep": idrep,
            "perms": perms,
            "pushw": pw,
            "crecip": crec_c,
            "crecip16": (16.0 * crec_c).astype(np.float32),
        })
    tables = (seg, counts, present, M, valid, w, crecip)
    return in_maps, tables


def combine(results, tables, n: int):
    total = np.float64(0.0)
    for r in results:
        total += np.asarray(r["res"], dtype=np.float64).sum()
    return np.float32(total / n)


_NC_CACHE: dict = {}


def _get_nc(n_core: int = NPC, reps: int = 1):
    key = ("v6", reps)
    if key not in _NC_CACHE:
        _NC_CACHE[key] = build_nc(n_core, reps=reps)
    return _NC_CACHE[key]


def kernel(outputs, labels, subbatch_indices):
    from concourse.bass_utils import run_bass_kernel_spmd

    outputs = np.asarray(outputs, dtype=np.float32)
    labels = np.asarray(labels, dtype=np.int32)
    subbatch_indices = np.asarray(subbatch_indices, dtype=np.int32)
    n = outputs.shape[0]

    nc = _get_nc()
    in_maps, tables = make_in_maps(outputs, labels, subbatch_indices)
    res = run_bass_kernel_spmd(nc, in_maps, list(range(NCORES)))
    return combine(res.results, tables, n)
